# revision 7
# baseline (speedup 1.0000x reference)
"""Trainium2 Bass kernel for nn_ConvAttention.

The reference computes:
    fx = conv1x1(x, wf) + bf          # [B,1,H,W]
    gx = conv1x1(x, wg) + bg
    hx = conv1x1(x, wh) + bh
    a  = softmax(fx @ gx, axis=1)     # axis of size 1 -> identically 1.0
    o  = (hx @ a) * x                 # hx @ ones = row-sum broadcast over W

Because the softmax is over a size-1 axis it is exactly 1.0 everywhere, so
    o[b,c,i,j] = s[b,i] * x[b,c,i,j]
    s[b,i]     = sum_c sum_k x[b,c,i,k] * wh[c] + W * bh
wf/bf/wg/bg do not affect the output. The kernel streams x once through
SBUF (read 16 MiB + write 16 MiB per core) - purely memory bound. The
fabric (SBUF AXI, ~435 GB/s/core) is the roofline; the goal is to keep the
two HWDGE queues (loads on Sync, stores on Scalar) saturated end to end.

Sharding: pure data parallel over batch; 4 batches per core on 8 cores.
Weights (wh, bh) replicated.

Per-core layout: for each (batch, c-chunk of 128, h-half of 32) an SBUF
tile [128 part = channels, 2048 free = 32*64 h,w] - contiguous 8 KiB per
partition in DRAM, 1 MiB per DMA.

Compute per (batch, h-half) group - kept OFF the DMA critical path:
  1. DVE tensor_reduce over w: [128, hh, 64] -> y[128, hh]   (all lanes)
  2. PE: 3 tiny matmuls into PSUM pb[128, hh]:
       bias:  lhsT=biasB[128,128] (W*bh/128 everywhere), rhs=ones[128,hh]
       chunk0/1: lhsT=whB[:,ch] ([128,128], every column = wh chunk),
                 rhs=y_ch  -> accumulates s[h] replicated on all 128 parts
     One matmul chain does contraction + partition-broadcast + bias.
  3. muls: ch0 on DVE reading pb straight from PSUM; ch1 on GpSimd
     reading an SBUF copy (GpSimd cannot touch PSUM).
  4. stores issued from the Scalar engine (its only job, so a store
     waiting on a mul never blocks unrelated work).
"""

from contextlib import ExitStack

import numpy as np

B, C, H, W = 32, 256, 64, 64
N_CORES = 8
BS = B // N_CORES  # batches per core

_CACHE = {}


def _split_multi_waits(nc, mybir):
    """Walrus codegen allows only one sync-wait slot on most instruction
    encodings ("Too many sync wait commands"). Tile's sem assigner sometimes
    attaches 2-3. Hoist the extras onto standalone EventSemaphore
    instructions immediately before, on the same engine - semantically
    identical since engines execute their stream in order."""
    n = 0
    for f in nc.m.functions:
        for bb in f.blocks:
            new_insts = []
            for inst in bb.instructions:
                si = inst.sync_info
                ow = list(si.on_wait) if si and si.on_wait else []
                if len(ow) > 1:
                    for wv in ow[:-1]:
                        n += 1
                        evs = mybir.InstEventSemaphore(
                            name=f"evs_split_{n}",
                            ins=[],
                            outs=[],
                            engine=inst.engine,
                            bass_nofuse=True,
                            sync_info=mybir.SyncInfo(on_wait=[wv], on_update=[]),
                        )
                        nc.register_instruction(evs, overwrite=True)
                        new_insts.append(evs)
                    inst.sync_info = mybir.SyncInfo(
                        on_wait=[ow[-1]],
                        on_update=list(si.on_update) if si.on_update else [],
                    )
                new_insts.append(inst)
            bb.instructions = new_insts
    return n


def _build(bs, c, h, w):
    import concourse.bass as bass
    import concourse.tile as tile
    from concourse import mybir

    f32 = mybir.dt.float32
    P = 128
    n_ch = c // P
    assert c % P == 0
    n_half = 2 if h % 2 == 0 else 1
    hh = h // n_half
    fh = hh * w  # free elems per tile

    nc = bass.Bass("TRN2", target_bir_lowering=False, debug=False)
    x = nc.dram_tensor("x", [bs, c, h, w], f32, kind="ExternalInput").ap()
    wh = nc.dram_tensor("wh", [c], f32, kind="ExternalInput").ap()
    bh = nc.dram_tensor("bh", [1], f32, kind="ExternalInput").ap()
    o = nc.dram_tensor("o", [bs, c, h, w], f32, kind="ExternalOutput").ap()

    X = mybir.AxisListType.X

    with tile.TileContext(nc) as tc, ExitStack() as ctx:
        consts = ctx.enter_context(tc.tile_pool(name="consts", bufs=1))
        xpool = ctx.enter_context(
            tc.tile_pool(name="xp", bufs=bs * n_ch * n_half)
        )
        ypool = ctx.enter_context(tc.tile_pool(name="yp", bufs=6))
        spool = ctx.enter_context(tc.tile_pool(name="sp", bufs=4))
        pbp = ctx.enter_context(tc.tile_pool(name="pb", bufs=4, space="PSUM"))

        # ---- x loads own the Sync HWDGE queue exclusively; the tiny const
        # DMAs ride the Scalar queue, which is idle until stores begin ----
        tiles = []  # (b, hf) -> [tile per ch]
        order = [(b, hf) for b in range(bs) for hf in range(n_half)]

        # A few early loads ride the Scalar queue too: until the first store
        # is ready (~t=17us) that queue is idle, and the SDMA engines
        # round-robin across queues, so loads keep the whole fabric while
        # stores are still compute-gated. Only loads that issue before the
        # first store may do this - later ones would sit behind store
        # instructions in the Scalar engine's FIFO.
        SCALAR_LOADS = {1, 3, 5}

        def load_tile(idx, b, hf, ch):
            xt = xpool.tile([P, fh], f32)
            eng = nc.scalar if idx in SCALAR_LOADS else nc.sync
            eng.dma_start(
                xt[:],
                x[
                    b, ch * P : (ch + 1) * P, hf * hh : (hf + 1) * hh
                ].rearrange("c h w -> c (h w)"),
            )
            return xt

        first = load_tile(0, *order[0], 0)

        # ---- constants (DMAs on Scalar queue; build ops on GpSimd so the
        # Vector engine's stream starts with real reduction work) ----
        # wh as [128, n_ch]: column j holds wh[j*128:(j+1)*128]
        wh_sb = consts.tile([P, n_ch], f32)
        nc.scalar.dma_start(wh_sb[:], wh.rearrange("(j p) -> p j", p=P))
        bh_sb = consts.tile([P, 1], f32)
        nc.scalar.dma_start(bh_sb[:], bh.to_broadcast((P, 1)))
        # whB[:, ch*128+m] = wh[ch*128+p] for every m: one matmul both
        # contracts over partitions and replicates the result on all 128
        biasW = consts.tile([P, 1], f32)
        nc.gpsimd.tensor_scalar_mul(biasW[:], bh_sb[:], float(w) / P)
        whB = consts.tile([P, n_ch * P], f32)
        for ch in range(n_ch):
            nc.gpsimd.tensor_copy(
                whB[:, ch * P : (ch + 1) * P],
                wh_sb[:, ch : ch + 1].broadcast_to((P, P)),
            )
        biasB = consts.tile([P, P], f32)
        nc.gpsimd.tensor_copy(biasB[:], biasW[:].broadcast_to((P, P)))
        ones_sb = consts.tile([P, hh], f32)
        nc.gpsimd.memset(ones_sb[:], 1.0)

        # ---- rest of the load stream ----
        for i, (b, hf) in enumerate(order):
            row = []
            for ch in range(n_ch):
                if i == 0 and ch == 0:
                    row.append(first)
                else:
                    row.append(load_tile(i * n_ch + ch, b, hf, ch))
            tiles.append(row)

        # ---- per-group pipeline ----
        for i, (b, hf) in enumerate(order):
            xts = tiles[i]
            # 1) w row-sums on DVE, all 128 lanes busy
            ys = []
            for ch in range(n_ch):
                y = ypool.tile([P, hh], f32)
                nc.vector.reduce_sum(
                    y[:], xts[ch][:].rearrange("c (h w) -> c h w", w=w), axis=X
                )
                ys.append(y)
            # 2) contraction + broadcast + bias in one PSUM accumulation
            pb = pbp.tile([P, hh], f32)
            nc.tensor.matmul(
                pb[:], lhsT=biasB[:], rhs=ones_sb[:], start=True, stop=False
            )
            for ch in range(n_ch):
                nc.tensor.matmul(
                    pb[:],
                    lhsT=whB[:, ch * P : (ch + 1) * P],
                    rhs=ys[ch][:],
                    start=False,
                    stop=(ch == n_ch - 1),
                )
            # 3) o = s * x in place, split between DVE (reads s straight
            # from PSUM) and GpSimd (reads the SBUF copy; no PSUM access).
            # DVE also carries the reduces, so it takes only ~0.4 tiles of
            # multiply work per group; GpSimd takes the rest - both engines
            # land at ~5.7us/group, under the ~9.9us steady load cadence.
            # For the final group split 50/50 to shorten the tail.
            s128 = spool.tile([P, hh], f32)
            nc.vector.tensor_copy(s128[:], pb[:])
            last = i == len(order) - 1
            # dve_cols[ch]: leading h-rows of tile ch multiplied on DVE
            if last:
                dve_rows = [hh // 2, hh // 2]
            else:
                dve_rows = [(hh * 7) // 16, 0]
            for ch in range(n_ch):
                xv = xts[ch][:].rearrange("c (h w) -> c h w", w=w)
                d = dve_rows[ch] if ch < len(dve_rows) else 0
                if d > 0:
                    nc.vector.tensor_mul(
                        xv[:, :d],
                        xv[:, :d],
                        pb[:, :d, None].broadcast_to((P, d, w)),
                    )
                if d < hh:
                    nc.gpsimd.tensor_mul(
                        xv[:, d:],
                        xv[:, d:],
                        s128[:, d:, None].broadcast_to((P, hh - d, w)),
                    )
                # The final stores drain through BOTH HWDGE rings: the Sync
                # ring is idle once the last load issued, and splitting the
                # tail packets across two rings halves the serial backlog on
                # the slowest SDMA engine (engine 15 straggles ~6us else).
                tail_store = i * n_ch + ch >= len(order) * n_ch - 3
                seng = nc.sync if tail_store else nc.scalar
                seng.dma_start(
                    o[
                        b, ch * P : (ch + 1) * P, hf * hh : (hf + 1) * hh
                    ].rearrange("c h w -> c (h w)"),
                    xts[ch][:],
                )
    _split_multi_waits(nc, mybir)
    return nc


def get_nc(bs=BS, c=C, h=H, w=W):
    key = (bs, c, h, w)
    if key not in _CACHE:
        _CACHE[key] = _build(bs, c, h, w)
    return _CACHE[key]


def kernel(x, wf, bf, wg, bg, wh, bh, **_unused):
    from concourse.bass_utils import run_bass_kernel_spmd

    x = np.ascontiguousarray(np.asarray(x, dtype=np.float32))
    wh = np.ascontiguousarray(np.asarray(wh, dtype=np.float32))
    bh = np.ascontiguousarray(np.asarray(bh, dtype=np.float32))

    in_maps = [
        {"x": x[k * BS : (k + 1) * BS], "wh": wh, "bh": bh} for k in range(N_CORES)
    ]
    # Tile scheduling is nondeterministic build-to-build and a rare schedule
    # can deadlock on hardware (NRT unrecoverable). Rebuilding produces a
    # fresh schedule, so retry with a clean build on any execution failure.
    last_err = None
    for attempt in range(3):
        try:
            nc = get_nc()
            res = run_bass_kernel_spmd(nc, in_maps, core_ids=list(range(N_CORES)))
            return np.concatenate(
                [res.results[k]["o"] for k in range(N_CORES)], axis=0
            )
        except Exception as e:  # rebuild with a new schedule and retry
            last_err = e
            _CACHE.clear()
    raise last_err


# revision 8
# speedup vs baseline: 1.1689x; 1.1689x over previous
"""Trainium2 Bass kernel for nn_ConvAttention.

The reference computes:
    fx = conv1x1(x, wf) + bf          # [B,1,H,W]
    gx = conv1x1(x, wg) + bg
    hx = conv1x1(x, wh) + bh
    a  = softmax(fx @ gx, axis=1)     # axis of size 1 -> identically 1.0
    o  = (hx @ a) * x                 # hx @ ones = row-sum broadcast over W

Because the softmax is over a size-1 axis it is exactly 1.0 everywhere, so
    o[b,c,i,j] = s[b,i] * x[b,c,i,j]
    s[b,i]     = sum_c sum_k x[b,c,i,k] * wh[c] + W * bh
wf/bf/wg/bg do not affect the output. The kernel streams x once through
SBUF (read 16 MiB + write 16 MiB per core) - purely memory bound. The
fabric (SBUF AXI, ~435 GB/s/core) is the roofline; the goal is to keep the
two HWDGE queues (loads on Sync, stores on Scalar) saturated end to end.

Sharding: pure data parallel over batch; 4 batches per core on 8 cores.
Weights (wh, bh) replicated.

Per-core layout: for each (batch, c-chunk of 128, h-half of 32) an SBUF
tile [128 part = channels, 2048 free = 32*64 h,w] - contiguous 8 KiB per
partition in DRAM, 1 MiB per DMA.

Compute per (batch, h-half) group - kept OFF the DMA critical path:
  1. DVE tensor_reduce over w: [128, hh, 64] -> y[128, hh]   (all lanes)
  2. PE: 3 tiny matmuls into PSUM pb[128, hh]:
       bias:  lhsT=biasB[128,128] (W*bh/128 everywhere), rhs=ones[128,hh]
       chunk0/1: lhsT=whB[:,ch] ([128,128], every column = wh chunk),
                 rhs=y_ch  -> accumulates s[h] replicated on all 128 parts
     One matmul chain does contraction + partition-broadcast + bias.
  3. muls: ch0 on DVE reading pb straight from PSUM; ch1 on GpSimd
     reading an SBUF copy (GpSimd cannot touch PSUM).
  4. stores issued from the Scalar engine (its only job, so a store
     waiting on a mul never blocks unrelated work).
"""

from contextlib import ExitStack

import numpy as np

B, C, H, W = 32, 256, 64, 64
N_CORES = 8
BS = B // N_CORES  # batches per core

_CACHE = {}


def _split_multi_waits(nc, mybir):
    """Walrus codegen allows only one sync-wait slot on most instruction
    encodings ("Too many sync wait commands"). Tile's sem assigner sometimes
    attaches 2-3. Hoist the extras onto standalone EventSemaphore
    instructions immediately before, on the same engine - semantically
    identical since engines execute their stream in order."""
    n = 0
    for f in nc.m.functions:
        for bb in f.blocks:
            new_insts = []
            for inst in bb.instructions:
                si = inst.sync_info
                ow = list(si.on_wait) if si and si.on_wait else []
                if len(ow) > 1:
                    for wv in ow[:-1]:
                        n += 1
                        evs = mybir.InstEventSemaphore(
                            name=f"evs_split_{n}",
                            ins=[],
                            outs=[],
                            engine=inst.engine,
                            bass_nofuse=True,
                            sync_info=mybir.SyncInfo(on_wait=[wv], on_update=[]),
                        )
                        nc.register_instruction(evs, overwrite=True)
                        new_insts.append(evs)
                    inst.sync_info = mybir.SyncInfo(
                        on_wait=[ow[-1]],
                        on_update=list(si.on_update) if si.on_update else [],
                    )
                new_insts.append(inst)
            bb.instructions = new_insts
    return n


def _build(bs, c, h, w):
    import concourse.bass as bass
    import concourse.tile as tile
    from concourse import mybir

    f32 = mybir.dt.float32
    P = 128
    n_ch = c // P
    assert c % P == 0
    n_half = 2 if h % 2 == 0 else 1
    hh = h // n_half
    fh = hh * w  # free elems per tile

    nc = bass.Bass("TRN2", target_bir_lowering=False, debug=False)
    x = nc.dram_tensor("x", [bs, c, h, w], f32, kind="ExternalInput").ap()
    wh = nc.dram_tensor("wh", [c], f32, kind="ExternalInput").ap()
    bh = nc.dram_tensor("bh", [1], f32, kind="ExternalInput").ap()
    o = nc.dram_tensor("o", [bs, c, h, w], f32, kind="ExternalOutput").ap()

    X = mybir.AxisListType.X

    with tile.TileContext(nc) as tc, ExitStack() as ctx:
        consts = ctx.enter_context(tc.tile_pool(name="consts", bufs=1))
        xpool = ctx.enter_context(tc.tile_pool(name="xp", bufs=bs * n_ch))
        ypool = ctx.enter_context(tc.tile_pool(name="yp", bufs=6))
        spool = ctx.enter_context(tc.tile_pool(name="sp", bufs=4))
        pbp = ctx.enter_context(tc.tile_pool(name="pb", bufs=4, space="PSUM"))

        # ---- constants: DMAs on the Scalar queue (idle until stores
        # begin), build ops on GpSimd ----
        # wh as [128, n_ch]: column j holds wh[j*128:(j+1)*128]
        wh_sb = consts.tile([P, n_ch], f32)
        nc.scalar.dma_start(wh_sb[:], wh.rearrange("(j p) -> p j", p=P))
        bh_sb = consts.tile([P, 1], f32)
        nc.scalar.dma_start(bh_sb[:], bh.to_broadcast((P, 1)))
        # whB[:, ch*128+m] = wh[ch*128+p] for every m: one matmul both
        # contracts over partitions and replicates the result on all 128
        biasW = consts.tile([P, 1], f32)
        nc.gpsimd.tensor_scalar_mul(biasW[:], bh_sb[:], float(w) / P)
        whB = consts.tile([P, n_ch * P], f32)
        for ch in range(n_ch):
            nc.gpsimd.tensor_copy(
                whB[:, ch * P : (ch + 1) * P],
                wh_sb[:, ch : ch + 1].broadcast_to((P, P)),
            )
        biasB = consts.tile([P, P], f32)
        nc.gpsimd.tensor_copy(biasB[:], biasW[:].broadcast_to((P, P)))
        ones_sb = consts.tile([P, h], f32)
        nc.gpsimd.memset(ones_sb[:], 1.0)

        # ---- the whole load stream is queued upfront: 8 DMAs of 2 MiB
        # (16 KiB contiguous per partition). They grab all 8 HWDGE sem
        # lanes first, so no load issue ever chains behind a store that is
        # still waiting on compute - the load queue can never starve. SBUF
        # comfortably holds all of x (16 MiB of 26), tiles are never
        # recycled. ----
        tiles = {}
        for b in range(bs):
            for ch in range(n_ch):
                xt = xpool.tile([P, h * w], f32)
                nc.sync.dma_start(
                    xt[:],
                    x[b, ch * P : (ch + 1) * P].rearrange("c h w -> c (h w)"),
                )
                tiles[(b, ch)] = xt

        # ---- per-batch pipeline ----
        for b in range(bs):
            xts = [tiles[(b, ch)] for ch in range(n_ch)]
            # 1) w row-sums on DVE, all 128 lanes busy: [128, h, w] -> [128, h]
            ys = []
            for ch in range(n_ch):
                y = ypool.tile([P, h], f32)
                nc.vector.reduce_sum(
                    y[:], xts[ch][:].rearrange("c (h w) -> c h w", w=w), axis=X
                )
                ys.append(y)
            # 2) contraction + broadcast + bias in one PSUM accumulation
            pb = pbp.tile([P, h], f32)
            nc.tensor.matmul(
                pb[:], lhsT=biasB[:], rhs=ones_sb[:], start=True, stop=False
            )
            for ch in range(n_ch):
                nc.tensor.matmul(
                    pb[:],
                    lhsT=whB[:, ch * P : (ch + 1) * P],
                    rhs=ys[ch][:],
                    start=False,
                    stop=(ch == n_ch - 1),
                )
            # 3) o = s * x in place per h-half (so 1 MiB stores flow as soon
            # as their half is scaled). DVE reads s straight from PSUM;
            # GpSimd reads the SBUF copy (no PSUM access). Steady batches:
            # DVE takes one of four quarters (it also carries the reduces),
            # GpSimd three. Final batch: two each, to shorten the tail.
            s128 = spool.tile([P, h], f32)
            nc.vector.tensor_copy(s128[:], pb[:])
            last = b == bs - 1
            dve_parts = (
                {(0, 0), (1, 0)} if last else {(0, 0)}
            )  # (ch, hf) quarters multiplied on DVE
            for ch in range(n_ch):
                xv = xts[ch][:].rearrange("c (h w) -> c h w", w=w)
                for hf in range(n_half):
                    lo, hi = hf * hh, (hf + 1) * hh
                    if (ch, hf) in dve_parts:
                        nc.vector.tensor_mul(
                            xv[:, lo:hi],
                            xv[:, lo:hi],
                            pb[:, lo:hi, None].broadcast_to((P, hh, w)),
                        )
                    else:
                        nc.gpsimd.tensor_mul(
                            xv[:, lo:hi],
                            xv[:, lo:hi],
                            s128[:, lo:hi, None].broadcast_to((P, hh, w)),
                        )
                    # Final-batch ch1 stores drain through the Sync ring:
                    # its load packets are long gone by then, so the tail
                    # splits across both rings (halves the backlog on the
                    # straggler SDMA engine 15).
                    seng = nc.sync if (last and ch == n_ch - 1) else nc.scalar
                    seng.dma_start(
                        o[
                            b, ch * P : (ch + 1) * P, lo:hi
                        ].rearrange("c h w -> c (h w)"),
                        xts[ch][:, lo * w : hi * w],
                    )
    _split_multi_waits(nc, mybir)
    return nc


def get_nc(bs=BS, c=C, h=H, w=W):
    key = (bs, c, h, w)
    if key not in _CACHE:
        _CACHE[key] = _build(bs, c, h, w)
    return _CACHE[key]


def kernel(x, wf, bf, wg, bg, wh, bh, **_unused):
    from concourse.bass_utils import run_bass_kernel_spmd

    x = np.ascontiguousarray(np.asarray(x, dtype=np.float32))
    wh = np.ascontiguousarray(np.asarray(wh, dtype=np.float32))
    bh = np.ascontiguousarray(np.asarray(bh, dtype=np.float32))

    in_maps = [
        {"x": x[k * BS : (k + 1) * BS], "wh": wh, "bh": bh} for k in range(N_CORES)
    ]
    # Tile scheduling is nondeterministic build-to-build and a rare schedule
    # can deadlock on hardware (NRT unrecoverable). Rebuilding produces a
    # fresh schedule, so retry with a clean build on any execution failure.
    last_err = None
    for attempt in range(3):
        try:
            nc = get_nc()
            res = run_bass_kernel_spmd(nc, in_maps, core_ids=list(range(N_CORES)))
            return np.concatenate(
                [res.results[k]["o"] for k in range(N_CORES)], axis=0
            )
        except Exception as e:  # rebuild with a new schedule and retry
            last_err = e
            _CACHE.clear()
    raise last_err


# revision 12
# speedup vs baseline: 1.2264x; 1.0492x over previous
"""Trainium2 Bass kernel for nn_ConvAttention.

The reference computes:
    fx = conv1x1(x, wf) + bf          # [B,1,H,W]
    gx = conv1x1(x, wg) + bg
    hx = conv1x1(x, wh) + bh
    a  = softmax(fx @ gx, axis=1)     # axis of size 1 -> identically 1.0
    o  = (hx @ a) * x                 # hx @ ones = row-sum broadcast over W

Because the softmax is over a size-1 axis it is exactly 1.0 everywhere, so
    o[b,c,i,j] = s[b,i] * x[b,c,i,j]
    s[b,i]     = sum_c sum_k x[b,c,i,k] * wh[c] + W * bh
wf/bf/wg/bg do not affect the output. The kernel streams x once through
SBUF (read 16 MiB + write 16 MiB per core) - purely memory bound. The
fabric (SBUF AXI, ~435 GB/s/core) is the roofline; the goal is to keep the
two HWDGE queues (loads on Sync, stores on Scalar) saturated end to end.

Sharding: pure data parallel over batch; 4 batches per core on 8 cores.
Weights (wh, bh) replicated.

Per-core layout: for each (batch, c-chunk of 128, h-half of 32) an SBUF
tile [128 part = channels, 2048 free = 32*64 h,w] - contiguous 8 KiB per
partition in DRAM, 1 MiB per DMA.

Compute per (batch, h-half) group - kept OFF the DMA critical path:
  1. DVE tensor_reduce over w: [128, hh, 64] -> y[128, hh]   (all lanes)
  2. PE: 3 tiny matmuls into PSUM pb[128, hh]:
       bias:  lhsT=biasB[128,128] (W*bh/128 everywhere), rhs=ones[128,hh]
       chunk0/1: lhsT=whB[:,ch] ([128,128], every column = wh chunk),
                 rhs=y_ch  -> accumulates s[h] replicated on all 128 parts
     One matmul chain does contraction + partition-broadcast + bias.
  3. muls: ch0 on DVE reading pb straight from PSUM; ch1 on GpSimd
     reading an SBUF copy (GpSimd cannot touch PSUM).
  4. stores issued from the Scalar engine (its only job, so a store
     waiting on a mul never blocks unrelated work).
"""

from contextlib import ExitStack

import numpy as np

B, C, H, W = 32, 256, 64, 64
N_CORES = 8
BS = B // N_CORES  # batches per core

_CACHE = {}


def _split_multi_waits(nc, mybir):
    """Walrus codegen allows only one sync-wait slot on most instruction
    encodings ("Too many sync wait commands"). Tile's sem assigner sometimes
    attaches 2-3. Hoist the extras onto standalone EventSemaphore
    instructions immediately before, on the same engine - semantically
    identical since engines execute their stream in order."""
    n = 0
    for f in nc.m.functions:
        for bb in f.blocks:
            new_insts = []
            for inst in bb.instructions:
                si = inst.sync_info
                ow = list(si.on_wait) if si and si.on_wait else []
                if len(ow) > 1:
                    for wv in ow[:-1]:
                        n += 1
                        evs = mybir.InstEventSemaphore(
                            name=f"evs_split_{n}",
                            ins=[],
                            outs=[],
                            engine=inst.engine,
                            bass_nofuse=True,
                            sync_info=mybir.SyncInfo(on_wait=[wv], on_update=[]),
                        )
                        nc.register_instruction(evs, overwrite=True)
                        new_insts.append(evs)
                    inst.sync_info = mybir.SyncInfo(
                        on_wait=[ow[-1]],
                        on_update=list(si.on_update) if si.on_update else [],
                    )
                new_insts.append(inst)
            bb.instructions = new_insts
    return n


def _build(bs, c, h, w):
    import concourse.bass as bass
    import concourse.tile as tile
    from concourse import mybir

    f32 = mybir.dt.float32
    P = 128
    n_ch = c // P
    assert c % P == 0
    n_half = 2 if h % 2 == 0 else 1
    hh = h // n_half
    fh = hh * w  # free elems per tile

    nc = bass.Bass("TRN2", target_bir_lowering=False, debug=False)
    x = nc.dram_tensor("x", [bs, c, h, w], f32, kind="ExternalInput").ap()
    wh = nc.dram_tensor("wh", [c], f32, kind="ExternalInput").ap()
    bh = nc.dram_tensor("bh", [1], f32, kind="ExternalInput").ap()
    o = nc.dram_tensor("o", [bs, c, h, w], f32, kind="ExternalOutput").ap()

    X = mybir.AxisListType.X

    with tile.TileContext(nc) as tc, ExitStack() as ctx:
        consts = ctx.enter_context(tc.tile_pool(name="consts", bufs=1))
        xpool = ctx.enter_context(tc.tile_pool(name="xp", bufs=bs * n_ch))
        ypool = ctx.enter_context(tc.tile_pool(name="yp", bufs=6))
        spool = ctx.enter_context(tc.tile_pool(name="sp", bufs=4))
        pbp = ctx.enter_context(tc.tile_pool(name="pb", bufs=4, space="PSUM"))

        # ---- constants: their DMAs have tiny 4-byte descriptors (HBM
        # read-modify-write, ~15-20us completion!) so they go on the SWDGE
        # queue - separate DMASW sem lanes, can never block the x stream's
        # HWDGE lanes. bh is replicated on-chip instead of a broadcast DMA.
        # Build ops on GpSimd. ----
        # wh as [128, n_ch]: column j holds wh[j*128:(j+1)*128]
        wh_sb = consts.tile([P, n_ch], f32)
        nc.gpsimd.dma_start(wh_sb[:], wh.rearrange("(j p) -> p j", p=P))
        bh_flat = consts.tile([1, 1], f32)
        nc.gpsimd.dma_start(bh_flat[:], bh[None, :])
        # bias enters pb via a K=1 matmul: lhsT = [1,128] of W*bh, rhs =
        # [1,h] of ones -> out[m,n] = W*bh on every partition. Only
        # single-partition operands needed, no broadcast DMA.
        bh_row = consts.tile([1, P], f32)
        nc.gpsimd.tensor_scalar_mul(
            bh_row[:1, :], bh_flat[:1, :1].broadcast_to((1, P)), float(w)
        )
        ones_row = consts.tile([1, h], f32)
        nc.gpsimd.memset(ones_row[:1, :], 1.0)
        # whB[:, ch*128+m] = wh[ch*128+p] for every m: one matmul both
        # contracts over partitions and replicates the result on all 128
        whB = consts.tile([P, n_ch * P], f32)
        for ch in range(n_ch):
            nc.gpsimd.tensor_copy(
                whB[:, ch * P : (ch + 1) * P],
                wh_sb[:, ch : ch + 1].broadcast_to((P, P)),
            )

        # ---- the whole load stream is queued upfront: 8 DMAs of 2 MiB
        # (16 KiB contiguous per partition). They grab all 8 HWDGE sem
        # lanes first, so no load issue ever chains behind a store that is
        # still waiting on compute - the load queue can never starve. SBUF
        # comfortably holds all of x (16 MiB of 26), tiles are never
        # recycled. ----
        tiles = {}
        with tc.high_priority():
            for b in range(bs):
                for ch in range(n_ch):
                    xt = xpool.tile([P, h * w], f32)
                    nc.sync.dma_start(
                        xt[:],
                        x[b, ch * P : (ch + 1) * P].rearrange(
                            "c h w -> c (h w)"
                        ),
                    )
                    tiles[(b, ch)] = xt

        # ---- per-batch pipeline ----
        for b in range(bs):
            xts = [tiles[(b, ch)] for ch in range(n_ch)]
            # 1) w row-sums on DVE, all 128 lanes busy: [128, h, w] -> [128, h]
            ys = []
            for ch in range(n_ch):
                y = ypool.tile([P, h], f32)
                nc.vector.reduce_sum(
                    y[:], xts[ch][:].rearrange("c (h w) -> c h w", w=w), axis=X
                )
                ys.append(y)
            # 2) contraction + broadcast + bias in one PSUM accumulation
            pb = pbp.tile([P, h], f32)
            nc.tensor.matmul(
                pb[:], lhsT=bh_row[:1, :], rhs=ones_row[:1, :],
                start=True, stop=False,
            )
            for ch in range(n_ch):
                nc.tensor.matmul(
                    pb[:],
                    lhsT=whB[:, ch * P : (ch + 1) * P],
                    rhs=ys[ch][:],
                    start=False,
                    stop=(ch == n_ch - 1),
                )
            # 3) o = s * x in place per h-half (so 1 MiB stores flow as soon
            # as their half is scaled). DVE reads s straight from PSUM;
            # GpSimd reads the SBUF copy (no PSUM access). Steady batches:
            # DVE takes one of four quarters (it also carries the reduces),
            # GpSimd three. Final batch: two each, to shorten the tail.
            s128 = spool.tile([P, h], f32)
            nc.vector.tensor_copy(s128[:], pb[:])
            last = b == bs - 1
            dve_parts = (
                {(0, 0), (1, 0)} if last else {(0, 0)}
            )  # (ch, hf) quarters multiplied on DVE
            for ch in range(n_ch):
                xv = xts[ch][:].rearrange("c (h w) -> c h w", w=w)
                for hf in range(n_half):
                    lo, hi = hf * hh, (hf + 1) * hh
                    if (ch, hf) in dve_parts:
                        nc.vector.tensor_mul(
                            xv[:, lo:hi],
                            xv[:, lo:hi],
                            pb[:, lo:hi, None].broadcast_to((P, hh, w)),
                        )
                    else:
                        nc.gpsimd.tensor_mul(
                            xv[:, lo:hi],
                            xv[:, lo:hi],
                            s128[:, lo:hi, None].broadcast_to((P, hh, w)),
                        )
                    # Final-batch ch1 stores drain through the Sync ring:
                    # its load packets are long gone by then, so the tail
                    # splits across both rings (halves the backlog on the
                    # straggler SDMA engine 15).
                    seng = nc.sync if (last and ch == n_ch - 1) else nc.scalar
                    seng.dma_start(
                        o[
                            b, ch * P : (ch + 1) * P, lo:hi
                        ].rearrange("c h w -> c (h w)"),
                        xts[ch][:, lo * w : hi * w],
                    )
    _split_multi_waits(nc, mybir)
    return nc


def get_nc(bs=BS, c=C, h=H, w=W):
    key = (bs, c, h, w)
    if key not in _CACHE:
        _CACHE[key] = _build(bs, c, h, w)
    return _CACHE[key]


def kernel(x, wf, bf, wg, bg, wh, bh, **_unused):
    from concourse.bass_utils import run_bass_kernel_spmd

    x = np.ascontiguousarray(np.asarray(x, dtype=np.float32))
    wh = np.ascontiguousarray(np.asarray(wh, dtype=np.float32))
    bh = np.ascontiguousarray(np.asarray(bh, dtype=np.float32))

    in_maps = [
        {"x": x[k * BS : (k + 1) * BS], "wh": wh, "bh": bh} for k in range(N_CORES)
    ]
    # Tile scheduling is nondeterministic build-to-build and a rare schedule
    # can deadlock on hardware (NRT unrecoverable). Rebuilding produces a
    # fresh schedule, so retry with a clean build on any execution failure.
    last_err = None
    for attempt in range(3):
        try:
            nc = get_nc()
            res = run_bass_kernel_spmd(nc, in_maps, core_ids=list(range(N_CORES)))
            return np.concatenate(
                [res.results[k]["o"] for k in range(N_CORES)], axis=0
            )
        except Exception as e:  # rebuild with a new schedule and retry
            last_err = e
            _CACHE.clear()
    raise last_err
